# revision 10
# baseline (speedup 1.0000x reference)
"""Trainium2 Bass kernel for nn_MemResProjections — v3.

v2 -> v3 changes (from v2 trace: ACT 60% busy = bottleneck, PE 85us gaps,
first matmul at t=35us, fp8 projections/scores fail accuracy):
  * G-trick: logits = n_s^T G n_t with G = diag(nw) q_w^T k_w diag(nw)
    precomputed on host -> the whole q projection disappears; scores
    consume normed^T directly (rhs) against kTg = G @ n~ (lhsT).
  * dtypes: bf16 projections + scores (accuracy), fp8 DoubleRow h~ path
    with a first-order residual on v (v8 + vr8), bf16 memory-v path
    (fp8 vmem alone cost 1.4e-2 rel err).  Predicted rel err ~8e-3.
  * norm_w folded into the output combine (v holds raw normed rows).
  * ACT batching: paired exp tiles [128,512], single-op gate eviction
    chain per stripe ([128,1024] psum), single-op hsb eviction.
  * PE warmup matmuls + x-stripe DMAs issued before weight DMAs.

Sharding (unchanged from v2): 8 cores = (batch, parity); core owns the
odd/even 128-stripes of its batch, all tensors in own-first coordinates;
causal envelope per 256-query slot is position ranges {0..2k+1} u
{8..9+2k} + mem on every core (uniform SPMD), fringe masks are per-core
DRAM data.
"""
import numpy as np
import ml_dtypes

import concourse.bass as bass
import concourse.mybir as mybir
import concourse.tile as tile
from concourse.bass_utils import run_bass_kernel_spmd

F32 = mybir.dt.float32
BF16 = mybir.dt.bfloat16
FP8 = mybir.dt.float8e4
FP16 = mybir.dt.float16
AFT = mybir.ActivationFunctionType
DR = mybir.MatmulPerfMode.DoubleRow

NP_BF16 = ml_dtypes.bfloat16
NP_FP8 = ml_dtypes.float8_e4m3

P = 128
H = 1024
NJ = 8
NS = 16
T_MEM = 64
TKV = 2048 + T_MEM
SCALE = 1.0 / 32.0
EXP_SHIFT = -2.0
EPS = 1e-6
NEG = -1.0e30

N_CORES = 8
B_FULL, S_FULL = 4, 2048

ENGINE_ATTR = {
    mybir.EngineType.PE: "tensor",
    mybir.EngineType.Activation: "scalar",
    mybir.EngineType.DVE: "vector",
    mybir.EngineType.Pool: "gpsimd",
    mybir.EngineType.SP: "sync",
}
DMA_OPS = ("InstDMACopy", "InstDMATranspose", "InstTensorLoad", "InstTensorSave",
           "InstCollectiveCompute")


def split_multi_waits(nc, limit=1, dma_limit=None):
    """This walrus build rejects engine instructions carrying more than one
    sem wait; hoist extras onto same-engine NOPs inserted just before."""
    n_split = 0
    for f in nc.m.functions:
        for blk in f.blocks:
            il = blk.instructions
            i = 0
            while i < len(il):
                ins = il[i]
                is_dma = type(ins).__name__ in DMA_OPS
                lim = dma_limit if is_dma else limit
                si = ins.sync_info
                waits = list(si.on_wait) if si is not None and si.on_wait else []
                if lim is not None and len(waits) > lim:
                    keep, extra = waits[:lim], waits[lim:]
                    si.on_wait.clear()
                    for w in keep:
                        si.on_wait.append(w)
                    eng = getattr(nc, ENGINE_ATTR[ins.engine])
                    for w in extra:
                        nop = eng.nop(nofuse=True, hint="wait_split")
                        nop.wait_op(bass.SemaphoreHandle(w.ant_name, w.id),
                                    w.wait_value, "sem-ge")
                        popped = nc.cur_bb.bb.instructions.pop()
                        assert popped.name == nop.ins.name
                        il.insert(i, nop.ins)
                        i += 1
                        n_split += 1
                i += 1
    return n_split


def build_nc():
    nc = bass.Bass()
    dp = lambda n, shp, dt=F32: nc.declare_dram_parameter(n, shp, dt,
                                                          isOutput=False)
    x_full = dp("x_full", [2048, H])
    Gw16 = dp("Gw16", [P, NJ, H], BF16)      # G.T as [p, j, o]
    gw16 = dp("gw16", [P, NJ, H], BF16)      # gate_w.T as [p, j, o]
    memx16 = dp("memx16", [P, NJ, T_MEM], BF16)
    vmem16 = dp("vmem16", [T_MEM, H], BF16)
    masks = dp("masks", [P, 4 * 256])
    w_bc32 = dp("w_bc32", [P, H])            # norm_w broadcast (output fold)
    b_bc = dp("b_bc", [P, H])
    id16 = dp("id16", [P, P], BF16)
    ones8 = dp("ones8", [P, 2, 1], FP8)
    out = nc.declare_dram_parameter("out", [1024, H], F32, isOutput=True)

    with tile.TileContext(nc) as tc:
        from contextlib import ExitStack
        with ExitStack() as ctx:
            const = ctx.enter_context(tc.tile_pool(name="const", bufs=1))
            res = ctx.enter_context(tc.tile_pool(name="res", bufs=1))

            # --- x stripes of the first group queue ahead of weights
            xt_first = []
            with tc.tile_pool(name="xfirst", bufs=1) as xf:
                id_t = const.tile([P, P], BF16)
                nc.sync.dma_start(out=id_t[:], in_=id16[:])
                for i in range(2):
                    xt = xf.tile([P, H], F32, tag=f"x{i}", name=f"x{i}")
                    nc.sync.dma_start(out=xt[:],
                                      in_=x_full[i * P:(i + 1) * P, :])
                    xt_first.append(xt)
                mask_t = const.tile([P, 4 * 256], F32)
                nc.sync.dma_start(out=mask_t[:], in_=masks[:])
                w_bc_t = const.tile([P, H], F32)
                nc.sync.dma_start(out=w_bc_t[:], in_=w_bc32[:])
                b_bc_t = const.tile([P, H], F32)
                nc.sync.dma_start(out=b_bc_t[:], in_=b_bc[:])
                ones_t = const.tile([P, 2, 1], FP8)
                nc.sync.dma_start(out=ones_t[:], in_=ones8[:])
                ones16_t = const.tile([T_MEM, 1], BF16)
                nc.vector.memset(ones16_t[:], 1.0)
                std_all = const.tile([P, NS], F32)
                rstd_all = const.tile([P, NS], F32)
                rden = const.tile([P, NJ], F32)
                eps_t = const.tile([P, 1], F32)
                nc.vector.memset(eps_t[:], EPS)
                shift_t = const.tile([P, 1], F32)
                nc.vector.memset(shift_t[:], EXP_SHIFT)
                wup = const.tile([P, P], F32)

                vmem_t = res.tile([T_MEM, H], BF16)
                nT = res.tile([P, NJ, TKV], BF16)
                v8 = res.tile([P, NS, H], FP8)
                vr8 = res.tile([P, NS, H], FP8)
                kTg = res.tile([P, NJ, TKV], BF16)
                alpha = res.tile([P, NJ, H], FP16)



                # ============ phase A: pipelined norm/transpose/projections
                with tc.tile_pool(name="aw", bufs=1) as aw, \
                     tc.tile_pool(name="aps", bufs=2, space="PSUM") as aps, \
                     tc.tile_pool(name="apj", bufs=2, space="PSUM") as apj, \
                     tc.tile_pool(name="apg", bufs=2, space="PSUM") as apg:
                    gw_t = aw.tile([P, NJ, H], BF16, tag="gwt", bufs=1)
                    Gw_t = aw.tile([P, NJ, H], BF16, tag="Gwt", bufs=1)
                    # PE warmup: keep HAM busy while stripe 0 stats run
                    tpw = apj.tile([P, 512], F32, tag="pj")
                    for w in range(64):
                        nc.tensor.matmul(tpw[:, 0:P], id_t[:], id_t[:],
                                         start=True, stop=True)
                    nc.vector.tensor_copy(wup[:], tpw[:, 0:P])

                    # prefetch stripes 2-5 ahead of the big weight DMAs,
                    # then queue weights in first-use order
                    xts_pre = {}
                    for i in range(2, 6):
                        xt = aw.tile([P, H], F32, tag="xt", bufs=4)
                        nc.sync.dma_start(out=xt[:],
                                          in_=x_full[i * P:(i + 1) * P, :])
                        xts_pre[i] = xt
                    nc.sync.dma_start(out=Gw_t[:], in_=Gw16[:])
                    nc.sync.dma_start(out=gw_t[:], in_=gw16[:])
                    nc.sync.dma_start(out=vmem_t[:], in_=vmem16[:])
                    nc.sync.dma_start(out=nT[:, :, 2048:TKV], in_=memx16[:])

                    def emit_stripes(g):
                        xts = {}
                        for i in range(4 * g, 4 * g + 4):
                            if i < 2:
                                xt = xt_first[i]
                            elif i in xts_pre:
                                xt = xts_pre.pop(i)
                            else:
                                xt = aw.tile([P, H], F32, tag="xt", bufs=4)
                                nc.sync.dma_start(
                                    out=xt[:], in_=x_full[i * P:(i + 1) * P, :])
                            xts[i] = xt
                            sq = aw.tile([P, H], BF16, tag="sq", bufs=2)
                            ss = aw.tile([P, 1], F32, tag="ss", bufs=2)
                            nc.scalar.activation(sq[:], xt[:], AFT.Square,
                                                 accum_out=ss[:])
                            nc.scalar.activation(std_all[:, i:i + 1], ss[:],
                                                 AFT.Sqrt, scale=1.0 / H,
                                                 bias=eps_t[:])
                            nc.vector.reciprocal(rstd_all[:, i:i + 1],
                                                 std_all[:, i:i + 1])
                        for i in range(4 * g, 4 * g + 4):
                            xt = xts[i]
                            nrm = aw.tile([P, H], BF16, tag="nrm", bufs=2)
                            nc.scalar.activation(nrm[:], xt[:], AFT.Copy,
                                                 scale=rstd_all[:, i:i + 1])
                            nc.scalar.activation(v8[:, i, :], xt[:],
                                                 AFT.Copy,
                                                 scale=rstd_all[:, i:i + 1])
                            nc.vector.tensor_sub(vr8[:, i, :], nrm[:],
                                                 v8[:, i, :])
                            for half in range(2):
                                tp = aps.tile([P, 512], F32, tag="tp")
                                for jj in range(4):
                                    j = half * 4 + jj
                                    nc.tensor.matmul(
                                        tp[:, jj * P:(jj + 1) * P],
                                        nrm[:, j * P:(j + 1) * P], id_t[:],
                                        start=True, stop=True)
                                j0 = half * 4
                                nc.vector.tensor_copy(
                                    nT[:, j0:j0 + 4, i * P:(i + 1) * P], tp[:])

                    def emit_proj(g):
                        c0, c1 = g * 512, (g + 1) * 512
                        # kTg for this column group (bf16)
                        for m in range(NJ):
                            pk = apj.tile([P, 512], F32, tag="pj")
                            for j in range(NJ):
                                nc.tensor.matmul(
                                    pk[:], Gw_t[:, j, m * P:(m + 1) * P],
                                    nT[:, j, c0:c1],
                                    start=(j == 0), stop=(j == NJ - 1))
                            nc.vector.tensor_copy(kTg[:, m, c0:c1], pk[:])
                        if g < 2:
                            # gate for the 4 own stripes of this group
                            for si in range(4 * g, 4 * g + 4):
                                pg = apg.tile([P, 1024], F32, tag="pg")
                                for oc in range(2):
                                    for j in range(NJ):
                                        nc.tensor.matmul(
                                            pg[:, oc * 512:(oc + 1) * 512],
                                            nT[:, j, si * P:(si + 1) * P],
                                            gw_t[:, j, oc * 512:(oc + 1) * 512],
                                            start=(j == 0), stop=(j == NJ - 1))
                                glf = aw.tile([P, H], F32, tag="glf", bufs=2)
                                nc.vector.scalar_tensor_tensor(
                                    glf[:], pg[:], std_all[:, si:si + 1],
                                    b_bc_t[:], mybir.AluOpType.mult,
                                    mybir.AluOpType.add)
                                nc.scalar.activation(alpha[:, si, :], glf[:],
                                                     AFT.Sigmoid)
                    emit_stripes(0)
                    tpw2 = apj.tile([P, 512], F32, tag="pj")
                    for w in range(60):
                        nc.tensor.matmul(tpw2[:, 0:P], id_t[:], id_t[:],
                                         start=True, stop=True)
                    nc.vector.tensor_copy(wup[:], tpw2[:, 0:P])
                    first = True
                    for g in range(1, 4):
                        emit_stripes(g)
                        if first:
                            # bridge the ACT-bound ramp so the PE clock
                            # stays at 8/8 into the first projection
                            tpw3 = apj.tile([P, 512], F32, tag="pj")
                            for w in range(60):
                                nc.tensor.matmul(tpw3[:, 0:P], id_t[:],
                                                 id_t[:], start=True,
                                                 stop=True)
                            nc.vector.tensor_copy(wup[:], tpw3[:, 0:P])
                            first = False
                        emit_proj(g - 1)
                    emit_proj(3)
                    # memory kTg columns
                    for m in range(NJ):
                        pkm = apj.tile([P, T_MEM], F32, tag="pj")
                        for j in range(NJ):
                            nc.tensor.matmul(
                                pkm[:], Gw_t[:, j, m * P:(m + 1) * P],
                                nT[:, j, 2048:TKV],
                                start=(j == 0), stop=(j == NJ - 1))
                        nc.vector.tensor_copy(kTg[:, m, 2048:TKV], pkm[:])

            # ============ phase B: attention
            with tc.tile_pool(name="bw", bufs=1) as bw, \
                 tc.tile_pool(name="bexp", bufs=3) as bexp, \
                 tc.tile_pool(name="bps", bufs=2, space="PSUM") as bps, \
                 tc.tile_pool(name="bph", bufs=1, space="PSUM") as bph:
                for k in range(4):
                    q0 = k * 256
                    ph = [bph.tile([P, 1024], F32, tag=f"ph{sl}",
                                   name=f"ph{sl}") for sl in range(2)]
                    pd = [bph.tile([P, 1], F32, tag=f"pd{sl}", name=f"pd{sl}")
                          for sl in range(2)]
                    # jobs: own pairs, other pairs, then memory
                    jobs = ([("own", 2 * pi) for pi in range(k + 1)]
                            + [("oth", 8 + 2 * pi) for pi in range(k + 1)]
                            + [("mem", 16)])
                    aw_t, pre_t = [], []

                    def emit_slot_prep():
                        for sl in range(2):
                            sidx = 2 * k + sl
                            xs = bw.tile([P, H], F32, tag="xs", bufs=2)
                            nc.sync.dma_start(
                                out=xs[:],
                                in_=x_full[sidx * P:(sidx + 1) * P, :])
                            a32 = bw.tile([P, H], F32, tag="a32", bufs=2)
                            nc.scalar.activation(a32[:], alpha[:, sidx, :],
                                                 AFT.Copy)
                            aw32 = bw.tile([P, H], F32, tag="aw32", bufs=4)
                            nc.vector.tensor_mul(aw32[:], a32[:], w_bc_t[:])
                            aw_t.append(aw32)
                            xa = bw.tile([P, H], F32, tag="xa", bufs=2)
                            nc.vector.tensor_mul(xa[:], xs[:], a32[:])
                            pre = bw.tile([P, H], F32, tag="pre", bufs=4)
                            nc.vector.tensor_sub(pre[:], xs[:], xa[:])
                            pre_t.append(pre)
                    ets = []

                    def emit_scores(ji):
                        kind, pos = jobs[ji]
                        if kind == "mem":
                            et = bexp.tile([T_MEM, 256], BF16, tag="etm")
                            ps = bps.tile([P, 512], F32, tag="ps")
                            for j in range(NJ):
                                nc.tensor.matmul(
                                    ps[:T_MEM, 0:256],
                                    kTg[:, j, 2048:TKV],
                                    nT[:, j, q0:q0 + 256],
                                    start=(j == 0), stop=(j == NJ - 1))
                            nc.scalar.activation(et[:], ps[:T_MEM, 0:256],
                                                 AFT.Exp, scale=SCALE,
                                                 bias=shift_t[:T_MEM, :])
                            ets.append(et)
                            return
                        et = bexp.tile([P, 2, 256], FP8, tag="et")
                        ps = bps.tile([P, 512], F32, tag="ps")
                        for ko in range(2):
                            t = pos + ko
                            for j in range(NJ):
                                nc.tensor.matmul(
                                    ps[:, ko * 256:(ko + 1) * 256],
                                    kTg[:, j, t * P:(t + 1) * P],
                                    nT[:, j, q0:q0 + 256],
                                    start=(j == 0), stop=(j == NJ - 1))
                        if kind == "own" and pos == 2 * k:
                            nc.vector.tensor_add(ps[:], ps[:],
                                                 mask_t[:, 0:512])
                        elif kind == "oth" and pos == 8 + 2 * k:
                            nc.vector.tensor_add(ps[:], ps[:],
                                                 mask_t[:, 512:1024])
                        nc.scalar.activation(et[:, 0:2, :], ps[:], AFT.Exp,
                                             scale=SCALE, bias=shift_t[:])
                        ets.append(et)

                    def emit_hacc(ji):
                        kind, pos = jobs[ji]
                        first = ji == 0
                        last = ji == len(jobs) - 1
                        et = ets[ji]
                        for sl in range(2):
                            if kind == "mem":
                                lt = et[:, sl * P:(sl + 1) * P]
                                for hc in range(2):
                                    nc.tensor.matmul(
                                        ph[sl][:, hc * 512:(hc + 1) * 512], lt,
                                        vmem_t[:, hc * 512:(hc + 1) * 512],
                                        start=first, stop=last,
                                        skip_group_check=True)
                                nc.tensor.matmul(
                                    pd[sl][:], lt, ones16_t[:],
                                    start=first, stop=last,
                                    skip_group_check=True)
                            else:
                                lt = et[:, 0:2, sl * P:(sl + 1) * P]
                                for hc in range(2):
                                    nc.tensor.matmul(
                                        ph[sl][:, hc * 512:(hc + 1) * 512], lt,
                                        v8[:, pos:pos + 2,
                                           hc * 512:(hc + 1) * 512],
                                        start=first, stop=False,
                                        perf_mode=DR, skip_group_check=True)
                                    nc.tensor.matmul(
                                        ph[sl][:, hc * 512:(hc + 1) * 512], lt,
                                        vr8[:, pos:pos + 2,
                                            hc * 512:(hc + 1) * 512],
                                        start=False, stop=False,
                                        perf_mode=DR, skip_group_check=True)
                                nc.tensor.matmul(
                                    pd[sl][:], lt, ones_t[:, 0:2, :],
                                    start=first, stop=False,
                                    perf_mode=DR, skip_group_check=True)

                    for ji in range(len(jobs)):
                        emit_scores(ji)
                        if ji >= 1:
                            emit_hacc(ji - 1)
                        if ji == 1:
                            emit_slot_prep()
                    emit_hacc(len(jobs) - 1)

                    for sl in range(2):
                        sidx = 2 * k + sl
                        nc.vector.reciprocal(rden[:, sidx:sidx + 1], pd[sl][:])
                        hsb = bw.tile([P, H], F32, tag="hsb", bufs=2)
                        nc.vector.scalar_tensor_tensor(
                            hsb[:], ph[sl][:], rden[:, sidx:sidx + 1],
                            aw_t[sl][:], mybir.AluOpType.mult,
                            mybir.AluOpType.mult)
                        nc.vector.tensor_add(hsb[:], hsb[:], pre_t[sl][:])
                        nc.sync.dma_start(out=out[sidx * P:(sidx + 1) * P, :],
                                          in_=hsb[:])

    import os
    if os.environ.get("NO_WAIT_SPLIT") != "1":
        split_multi_waits(nc, limit=1, dma_limit=1)
    return nc


_NC_CACHE = None
_LAST_IN_MAPS = None


def _get_nc():
    global _NC_CACHE
    if _NC_CACHE is None:
        _NC_CACHE = build_nc()
    return _NC_CACHE


def _mk_masks(h):
    tri = np.where(np.arange(P)[None, :] >= np.arange(P)[:, None],
                   np.float32(0.0), np.float32(NEG)).astype(np.float32)
    Z = np.zeros((P, P), np.float32)
    NB = np.full((P, P), NEG, np.float32)
    m0 = np.concatenate([tri, Z], axis=1)
    m1 = np.concatenate([NB, tri], axis=1)
    if h == 0:
        m2 = np.concatenate([NB, Z], axis=1)
        m3 = np.concatenate([NB, NB], axis=1)
    else:
        m2 = np.concatenate([Z, Z], axis=1)
        m3 = np.concatenate([NB, Z], axis=1)
    return np.concatenate([m0, m1, m2, m3], axis=1)


def prepare_in_maps(hidden_states, memory_state, q_w, k_w, norm_w, gate_w,
                    gate_b):
    hidden_states = np.asarray(hidden_states, dtype=np.float32)
    memory_state = np.asarray(memory_state, dtype=np.float32)
    q_w = np.asarray(q_w, dtype=np.float32)
    k_w = np.asarray(k_w, dtype=np.float32)
    norm_w = np.asarray(norm_w, dtype=np.float32)
    gate_w = np.asarray(gate_w, dtype=np.float32)
    gate_b = np.asarray(gate_b, dtype=np.float32)

    def wrearrange(wT):   # [h, o] -> [p, j, o]
        return np.ascontiguousarray(wT.reshape(NJ, P, H).transpose(1, 0, 2))

    G = (q_w * norm_w[None, :]).T @ (k_w * norm_w[None, :])   # [h, h']
    Gw16 = wrearrange(np.ascontiguousarray(G.T)).astype(NP_BF16)
    gw16 = wrearrange(np.ascontiguousarray(gate_w.T)).astype(NP_BF16)
    w_bc32 = np.ascontiguousarray(np.broadcast_to(norm_w, (P, H)))
    b_bc = np.ascontiguousarray(np.broadcast_to(gate_b, (P, H)))
    id16 = np.eye(P, dtype=np.float32).astype(NP_BF16)
    ones8 = np.ones((P, 2, 1), np.float32).astype(NP_FP8)
    safe_nw = np.where(norm_w == 0.0, 1.0, norm_w)

    in_maps = []
    for c in range(N_CORES):
        b, h = divmod(c, 2)
        xs = hidden_states[b].reshape(NS, P, H)
        x_full = np.ascontiguousarray(
            np.concatenate([xs[h::2], xs[1 - h::2]], axis=0).reshape(2048, H))
        memb = memory_state[b]
        memx16 = np.ascontiguousarray(
            (memb / safe_nw[None, :]).T.reshape(NJ, P, T_MEM)
            .transpose(1, 0, 2)).astype(NP_BF16)
        in_maps.append({
            "x_full": x_full,
            "Gw16": Gw16, "gw16": gw16,
            "memx16": memx16,
            "vmem16": np.ascontiguousarray(memb).astype(NP_BF16),
            "masks": _mk_masks(h),
            "w_bc32": w_bc32, "b_bc": b_bc,
            "id16": id16, "ones8": ones8,
        })
    return in_maps


def kernel(**inputs):
    in_maps = prepare_in_maps(**inputs)
    global _LAST_IN_MAPS
    _LAST_IN_MAPS = in_maps
    nc = _get_nc()
    res = run_bass_kernel_spmd(nc, in_maps, list(range(N_CORES)))
    out = np.empty((B_FULL, S_FULL, H), dtype=np.float32)
    for c in range(N_CORES):
        b, h = divmod(c, 2)
        o = res.results[c]["out"].reshape(NJ, P, H)
        for i in range(NJ):
            out[b, (2 * i + h) * P:(2 * i + h + 1) * P] = o[i]
    return out
